# revision 30
# baseline (speedup 1.0000x reference)
"""Banded additive attention (width-128) on 8 TRN2 NeuronCores — raw Bass.

Problem: B=2, L=2048, F=128, U=32, WIDTH=128
  q = x@Wt + bh, k = x@Wx
  s_ij = Wa . tanh(q_i + k_j) + ba            (j in [i-64, i+63])
  e_ij = exp(sigmoid(s_ij)) * band * mask
  v_i  = sum_j e_ij x_j / (sum_j e_ij + 1e-7)

Sharding: core c handles batch c//4, queries [(c%4)*512, +512).  No
collectives.  Raw Bass; all synchronization is explicit standalone
wait_ge with hand-counted thresholds (walrus here rejects >1 sem wait
per instruction).

Per-core pipeline (partition p = 32*dm + u; block j in [0,32) covers
d = 4j+dm in [0,128); col i in [0,512) is the query):
  DVE  : arg[p,(j,i)] = q4[p,i] + K4[p, i+4j]      (bf16, 5 subchunks)
  ACT  : tanh(arg)                                 (the bulk)
  PE   : spB[64(j//16)+4(j%16)+dm, i] += block-diag W64 contraction
         over (dm,u) — all 32 matmuls accumulate into ONE psum bank
         (two 64-partition regions), so no placement DMAs and a single
         full-width exit.
  ACT  : cs0 = tanh(0.5*spB + 0.5*ba)  (= 2*sigmoid(s+ba)-1)
  PE   : 2-stage circular shear C[c,i] = sg[(c-i)%128, i]: stage A
         rotates by i%16 into bka laid out col=32v+a (16 contiguous
         col-class matmuls; the interpreter's psum pending-zero model
         requires contiguous matmul outputs).  Per-quad exits undo the
         permutation (2 on ACT, 2 on DVE), then stage B rotates by
         16*((i//16)%8) as 8 tiny contiguous matmuls per quad.
  ACT  : per-quad exp(0.5*x+0.5) = exp(sigmoid) -> cfin.
  DVE  : el quad = cfin quad * M (lower-triangle mask, c>=b); the
         complementary upper part never materializes because
         v = El.T @ (X[t]-X[t+1]) + C.T @ X[t+1]   (El+Eh = C exactly)
  PE   : vp[t] = El.T @ XD[t] + C_t.T @ X[t+1], four separate psum
         banks (a DVE read of one bank while PE accumulates another
         region of the SAME bank aborts the axon emulator); X carries
         a validity column so the denominator falls out of the matmul.
  DVE  : copy vp[t] -> ov slab;  two paired output DMAs;  host divides.
"""

import numpy as np
import ml_dtypes

B, L, F, U = 2, 2048, 128, 32
WIDTH = 128
EPS = 1e-7
NCORES = 8
QPC = (B * L) // NCORES          # 512 queries per core
NKEY = QPC + WIDTH               # 640 key rows per core
KW = NKEY                        # K4 sbuf width
BF16 = ml_dtypes.bfloat16

# subchunk partition of the 32 d-blocks (block = 4 consecutive d)
SUB_BLOCKS = [5, 7, 8, 8, 2, 2]
NSUB = len(SUB_BLOCKS)
SUB_START = [sum(SUB_BLOCKS[:i]) for i in range(NSUB)]
CUM_BLOCKS = [sum(SUB_BLOCKS[:i + 1]) for i in range(NSUB)]
MAXB = max(SUB_BLOCKS)

# shear rotation amounts: stage A = i%16, stage B = 16*((i//16)%8)
ROT_A = list(range(16))                 # 16 matrices (incl. identity)
ROT_B = [16 * w for w in range(1, 8)]   # 7 more (16..112); w=0 reuses R_0
ROTS = ROT_A + ROT_B                    # 23 matrices in the wr slab

W64_COLS = 16 * 64                      # 16 lhsT variants [128, 64]
ROT_COLS = len(ROTS) * 128
TRI_OFF = W64_COLS + ROT_COLS           # lower-triangle mask M
WR_COLS = TRI_OFF + 128
XS_COLS = 5 * 132 + 4 * 132             # X[0..4] then XD[0..3]

_built = None


def _build():
    import concourse.bass as bass
    import concourse.mybir as mybir

    f32 = mybir.dt.float32
    bf16 = mybir.dt.bfloat16
    Tanh = mybir.ActivationFunctionType.Tanh
    Exp = mybir.ActivationFunctionType.Exp

    nc = bass.Bass()

    qk_d = nc.dram_tensor("qk", [128, QPC + KW], bf16, kind="ExternalInput")
    wr_d = nc.dram_tensor("wr", [128, WR_COLS], bf16, kind="ExternalInput")
    xs_d = nc.dram_tensor("xs", [128, XS_COLS], bf16, kind="ExternalInput")
    ba_d = nc.dram_tensor("bat", [128, 2], f32, kind="ExternalInput")
    out_d = nc.dram_tensor("out", [128, 4 * 132], f32, kind="ExternalOutput")

    al = nc.alloc_sbuf_tensor
    qk = al("qks", [128, QPC + KW], bf16)
    wr = al("wrs", [128, WR_COLS], bf16)
    xs = al("xss", [128, XS_COLS], bf16)
    bat = al("bats", [128, 2], f32)
    arg = [al(f"arg{i}", [128, MAXB * 512], bf16) for i in range(2)]
    th = [al(f"th{i}", [128, MAXB * 512], bf16) for i in range(2)]
    cs0 = al("cs0", [128, QPC], bf16)
    cs1 = al("cs1", [128, QPC], bf16)
    cfin = al("cfin", [128, QPC], bf16)
    el = al("els", [128, QPC], bf16)
    ov = al("ovs", [128, 4 * 132], f32)

    ap = nc.alloc_psum_tensor
    spB = ap("spB", [128, QPC], f32)
    bka = ap("bka", [128, QPC], f32)
    bkb = ap("bkb", [128, QPC], f32)
    vp = [ap(f"vp{i}", [128, 132], f32) for i in range(4)]

    sem = nc.alloc_semaphore
    (sINQ, sINW, sINX, sINB, sADD, sTANH, sMM, sSE, sSH, sXD,
     sEXP, sTRI, sVMM, sEPI, sOUT) = (
        sem(n) for n in ("sINQ", "sINW", "sINX", "sINB", "sADD", "sTANH",
                         "sMM", "sSE", "sSH", "sXD",
                         "sEXP", "sTRI", "sVMM", "sEPI", "sOUT"))

    AP = bass.AP
    QKW = QPC + KW

    def q4ap(s):
        return AP(qk, 0, [[QKW, 128], [0, SUB_BLOCKS[s]], [1, QPC]])

    def k4ap(s):
        return AP(qk, QPC + 4 * SUB_START[s],
                  [[QKW, 128], [4, SUB_BLOCKS[s]], [1, QPC]])

    # stage-A col class v: cols {i : i%16 == v}
    def clsA(t, off):
        return AP(t, off, [[QPC, 128], [16, 32]])

    # exitA quad c: bka col 32v+a (a = 8c+w) -> cs1 col 128c+16w+v
    def exAsrc(c):
        return AP(bka, 8 * c, [[QPC, 128], [32, 16], [1, 8]])

    def exAdst(c):
        return AP(cs1, 128 * c, [[QPC, 128], [1, 16], [16, 8]])

    with nc.Block() as block:

        @block.sync
        def _(sync):
            sync.dma_start(qk[:, :], qk_d[:, :]).then_inc(sINQ, 16)
            sync.dma_start(wr[:, :], wr_d[:, :]).then_inc(sINW, 16)
            sync.dma_start(xs[:, :], xs_d[:, :]).then_inc(sINX, 16)
            sync.dma_start(bat[:, :], ba_d[:, :]).then_inc(sINB, 16)
            sync.wait_ge(sEPI, 2)
            sync.dma_start(out_d[:, 0:264], ov[:, 0:264]).then_inc(sOUT, 16)
            sync.wait_ge(sEPI, 4)
            sync.dma_start(out_d[:, 264:528],
                           ov[:, 264:528]).then_inc(sOUT, 16)

        @block.vector
        def _(vector):
            for s in range(NSUB):
                if s == 0:
                    vector.wait_ge(sINQ, 16)
                if s >= 2:
                    vector.wait_ge(sTANH, s - 1)   # arg[s%2] free
                vector.tensor_add(AP(arg[s % 2], 0,
                                     [[MAXB * 512, 128],
                                      [512, SUB_BLOCKS[s]], [1, 512]]),
                                  q4ap(s), k4ap(s)).then_inc(sADD, 1)
            # shear stage-A exit: one full-width permuted copy
            vector.wait_ge(sSH, 16)
            vector.tensor_copy(AP(cs1, 0, [[QPC, 128], [1, 16], [16, 32]]),
                               AP(bka, 0, [[QPC, 128], [32, 16], [1, 32]])
                               ).then_inc(sXD, 1)
            # triangle el = cfin * M, epilogue interleaved per quad
            for t in range(4):
                if t % 2 == 0:
                    vector.wait_ge(sEXP, t // 2 + 1)
                vector.tensor_tensor(el[:, 128 * t:128 * (t + 1)],
                                     cfin[:, 128 * t:128 * (t + 1)],
                                     wr[:, TRI_OFF:TRI_OFF + 128],
                                     op=mybir.AluOpType.mult).then_inc(sTRI, 1)
                if t >= 1:
                    vector.wait_ge(sVMM, t)
                    vector.tensor_copy(ov[:, 132 * (t - 1):132 * t],
                                       vp[t - 1][:, :]).then_inc(sEPI, 1)
            vector.wait_ge(sVMM, 4)
            vector.tensor_copy(ov[:, 396:528], vp[3][:, :]).then_inc(sEPI, 1)

        @block.scalar
        def _(scalar):
            for s in range(NSUB):
                scalar.wait_ge(sADD, s + 1)
                if s >= 2:
                    scalar.wait_ge(sMM, CUM_BLOCKS[s - 2])   # th[s%2] free
                w = 512 * SUB_BLOCKS[s]
                scalar.activation(th[s % 2][:, :w], arg[s % 2][:, :w],
                                  Tanh).then_inc(sTANH, 1)
            # score exit: tanh(0.5*s + 0.5*ba) = 2*sigmoid(s+ba) - 1
            scalar.wait_ge(sMM, 32)
            scalar.wait_ge(sINB, 16)
            scalar.activation(cs0[:, :], spB[:, :], Tanh,
                              bias=bat[:, 0:1], scale=0.5).then_inc(sSE, 1)
            # exp(0.5*x + 0.5) = exp(sigmoid) in halves; all of stage B
            # must be done first (same-bank concurrent access aborts)
            scalar.wait_ge(sSH, 48)
            for h in range(2):
                scalar.activation(cfin[:, 256 * h:256 * (h + 1)],
                                  bkb[:, 256 * h:256 * (h + 1)], Exp,
                                  bias=bat[:, 1:2], scale=0.5).then_inc(sEXP, 1)

        @block.tensor
        def _(tensor):
            tensor.wait_ge(sINW, 16)
            for j in range(32):
                s = next(i for i in range(NSUB) if j < CUM_BLOCKS[i])
                dgl = j - SUB_START[s]
                if dgl == 0:
                    tensor.wait_ge(sTANH, s + 1)
                v = j % 16
                r = j // 16
                tensor.matmul(spB[64 * r:64 * (r + 1), :],
                              wr[:, 64 * v:64 * (v + 1)],
                              th[s % 2][:, 512 * dgl:512 * (dgl + 1)],
                              start=(v == 0), stop=(v == 15)).then_inc(sMM, 1)
            # shear stage A: rotate col class v by v; contiguous psum block
            tensor.wait_ge(sSE, 1)
            for v in range(16):
                tensor.matmul(bka[:, 32 * v:32 * (v + 1)],
                              wr[:, W64_COLS + 128 * v:W64_COLS + 128 * (v + 1)],
                              clsA(cs0, v), start=True,
                              stop=True).then_inc(sSH, 1)
            # shear stage B per quad: rotate class w by 16w; all contiguous
            tensor.wait_ge(sXD, 1)
            for c in range(4):
                for w in range(8):
                    ri = w + 15 if w > 0 else 0      # R_16w slab index
                    off = 128 * c + 16 * w
                    tensor.matmul(bkb[:, off:off + 16],
                                  wr[:, W64_COLS + 128 * ri:W64_COLS + 128 * (ri + 1)],
                                  cs1[:, off:off + 16], start=True,
                                  stop=True).then_inc(sSH, 1)
            # v matmuls: vp[t] = El_t.T @ XD[t] + C_t.T @ X[t+1]
            tensor.wait_ge(sINX, 16)
            for t in range(4):
                tensor.wait_ge(sTRI, t + 1)
                tensor.matmul(vp[t][:, :],
                              el[:, 128 * t:128 * (t + 1)],
                              xs[:, 660 + 132 * t:660 + 132 * (t + 1)],
                              start=True, stop=False)
                tensor.matmul(vp[t][:, :],
                              cfin[:, 128 * t:128 * (t + 1)],
                              xs[:, 132 * (t + 1):132 * (t + 2)],
                              start=False, stop=True).then_inc(sVMM, 1)

        @block.gpsimd
        def _(gpsimd):
            gpsimd.wait_ge(sOUT, 32)

    nc.finalize()
    return nc


def _prep_inputs(x, mask, Wt, Wx, bh, Wa, ba):
    """Build the 8 per-core input maps (host-side sharding + projections)."""
    x64 = x.astype(np.float64)

    # W64 lhsT variants: variant v maps partition 32*dm+u -> out 4v+dm
    w64 = np.zeros((128, W64_COLS), np.float32)
    for v in range(16):
        for dm in range(4):
            w64[32 * dm:32 * (dm + 1), 64 * v + 4 * v + dm] = Wa[:, 0]
    # rotation matrices R_sh[p, m] = 1 iff m == (p + sh) % 128
    rot = np.zeros((128, ROT_COLS), np.float32)
    m = np.arange(128)
    for ri, sh in enumerate(ROTS):
        rot[(m - sh) % 128, 128 * ri + m] = 1.0
    # lower-triangle mask M[p, b] = 1 iff p >= b
    tri = (np.arange(128)[:, None] >= np.arange(128)[None, :]).astype(np.float32)
    wr = np.concatenate([w64, rot, tri], axis=1).astype(BF16)

    in_maps = []
    for c in range(NCORES):
        b = c // 4
        qs = (c % 4) * QPC
        q = (x64[b] @ Wt.astype(np.float64) + bh.astype(np.float64))
        k = (x64[b] @ Wx.astype(np.float64))
        qT = q[qs:qs + QPC].T.astype(np.float32)          # [32, 512]
        q4 = np.tile(qT, (4, 1))                          # [128, 512]
        lo = qs - 64
        s0, s1 = max(0, lo), min(L, lo + NKEY)
        kx = np.zeros((NKEY + 3, U), np.float64)
        kx[s0 - lo:s1 - lo] = k[s0:s1]
        K4 = np.zeros((128, KW), np.float32)
        for dm in range(4):
            K4[32 * dm:32 * (dm + 1), :] = kx[dm:dm + KW].T
        qkm = np.concatenate([q4, K4], axis=1).astype(BF16)

        mk = mask[b].astype(np.float32)
        xr = np.zeros((NKEY, F), np.float32)
        xr[s0 - lo:s1 - lo] = x[b, s0:s1] * mk[s0:s1, None]
        val = np.zeros(NKEY, np.float32)
        val[s0 - lo:s1 - lo] = mk[s0:s1]
        Xe = np.zeros((NKEY, 132), np.float32)
        Xe[:, :F] = xr
        Xe[:, F] = val
        xcols = [Xe[128 * t:128 * (t + 1)] for t in range(5)]
        xdcols = [xcols[t] - xcols[t + 1] for t in range(4)]
        xsl = np.concatenate(xcols + xdcols, axis=1).astype(BF16)

        bt = np.zeros((128, 2), np.float32)
        bt[:, 0] = 0.5 * float(ba[0])
        bt[:, 1] = 0.5
        in_maps.append({"qk": qkm, "wr": wr, "xs": xsl, "bat": bt})
    return in_maps


def kernel(x, mask, Wt, Wx, bh, Wa, ba, _want_results=False):
    global _built
    from concourse.bass_utils import run_bass_kernel_spmd
    x = np.asarray(x)
    mask = np.asarray(mask)
    Wt, Wx, bh, Wa, ba = (np.asarray(a) for a in (Wt, Wx, bh, Wa, ba))
    if _built is None:
        _built = _build()
    nc = _built
    in_maps = _prep_inputs(x, mask, Wt, Wx, bh, Wa, ba)
    res = run_bass_kernel_spmd(nc, in_maps, core_ids=list(range(NCORES)))
    v = np.zeros((B, L, F), np.float32)
    for c in range(NCORES):
        b = c // 4
        qs = (c % 4) * QPC
        o = res.results[c]["out"]                    # [128, 528]
        for t in range(4):
            blk = o[:, 132 * t:132 * (t + 1)]
            v[b, qs + 128 * t:qs + 128 * (t + 1)] = \
                blk[:, :F] / (blk[:, F:F + 1] + EPS)
    v *= mask.astype(np.float32)[:, :, None]
    if _want_results:
        return v, res
    return v


# revision 31
# speedup vs baseline: 1.0325x; 1.0325x over previous
"""Banded additive attention (width-128) on 8 TRN2 NeuronCores — raw Bass.

Problem: B=2, L=2048, F=128, U=32, WIDTH=128
  q = x@Wt + bh, k = x@Wx
  s_ij = Wa . tanh(q_i + k_j) + ba            (j in [i-64, i+63])
  e_ij = exp(sigmoid(s_ij)) * band * mask
  v_i  = sum_j e_ij x_j / (sum_j e_ij + 1e-7)

Sharding: core c handles batch c//4, queries [(c%4)*512, +512).  No
collectives.  Raw Bass; all synchronization is explicit standalone
wait_ge with hand-counted thresholds (walrus here rejects >1 sem wait
per instruction).

Per-core pipeline (partition p = 32*dm + u; block j in [0,32) covers
d = 4j+dm in [0,128); col i in [0,512) is the query):
  DVE  : arg[p,(j,i)] = q4[p,i] + K4[p, i+4j]      (bf16, 5 subchunks)
  ACT  : tanh(arg)                                 (the bulk)
  PE   : spB[64(j//16)+4(j%16)+dm, i] += block-diag W64 contraction
         over (dm,u) — all 32 matmuls accumulate into ONE psum bank
         (two 64-partition regions), so no placement DMAs and a single
         full-width exit.
  ACT  : cs0 = tanh(0.5*spB + 0.5*ba)  (= 2*sigmoid(s+ba)-1)
  PE   : 2-stage circular shear C[c,i] = sg[(c-i)%128, i]: stage A
         rotates by i%16 into bka laid out col=32v+a (16 contiguous
         col-class matmuls; the interpreter's psum pending-zero model
         requires contiguous matmul outputs).  Per-quad exits undo the
         permutation (2 on ACT, 2 on DVE), then stage B rotates by
         16*((i//16)%8) as 8 tiny contiguous matmuls per quad.
  ACT  : per-quad exp(0.5*x+0.5) = exp(sigmoid) -> cfin.
  DVE  : el quad = cfin quad * M (lower-triangle mask, c>=b); the
         complementary upper part never materializes because
         v = El.T @ (X[t]-X[t+1]) + C.T @ X[t+1]   (El+Eh = C exactly)
  PE   : vp[t] = El.T @ XD[t] + C_t.T @ X[t+1], four separate psum
         banks (a DVE read of one bank while PE accumulates another
         region of the SAME bank aborts the axon emulator); X carries
         a validity column so the denominator falls out of the matmul.
  DVE  : copy vp[t] -> ov slab;  two paired output DMAs;  host divides.
"""

import numpy as np
import ml_dtypes

B, L, F, U = 2, 2048, 128, 32
WIDTH = 128
EPS = 1e-7
NCORES = 8
QPC = (B * L) // NCORES          # 512 queries per core
NKEY = QPC + WIDTH               # 640 key rows per core
KW = NKEY                        # K4 sbuf width
BF16 = ml_dtypes.bfloat16

# subchunk partition of the 32 d-blocks (block = 4 consecutive d)
SUB_BLOCKS = [5, 7, 8, 8, 2, 2]
NSUB = len(SUB_BLOCKS)
SUB_START = [sum(SUB_BLOCKS[:i]) for i in range(NSUB)]
CUM_BLOCKS = [sum(SUB_BLOCKS[:i + 1]) for i in range(NSUB)]
MAXB = max(SUB_BLOCKS)

# shear rotation amounts: stage A = i%16, stage B = 16*((i//16)%8)
ROT_A = list(range(16))                 # 16 matrices (incl. identity)
ROT_B = [16 * w for w in range(1, 8)]   # 7 more (16..112); w=0 reuses R_0
ROTS = ROT_A + ROT_B                    # 23 matrices in the wr slab

W64_COLS = 16 * 64                      # 16 lhsT variants [128, 64]
ROT_COLS = len(ROTS) * 128
TRI_OFF = W64_COLS + ROT_COLS           # lower-triangle mask M
WR_COLS = TRI_OFF + 128
XS_COLS = 5 * 132 + 4 * 132             # X[0..4] then XD[0..3]

_built = None


def _build():
    import concourse.bass as bass
    import concourse.mybir as mybir

    f32 = mybir.dt.float32
    bf16 = mybir.dt.bfloat16
    Tanh = mybir.ActivationFunctionType.Tanh
    Exp = mybir.ActivationFunctionType.Exp

    nc = bass.Bass()

    qk_d = nc.dram_tensor("qk", [128, QPC + KW], bf16, kind="ExternalInput")
    wr_d = nc.dram_tensor("wr", [128, WR_COLS], bf16, kind="ExternalInput")
    xs_d = nc.dram_tensor("xs", [128, XS_COLS], bf16, kind="ExternalInput")
    ba_d = nc.dram_tensor("bat", [128, 2], f32, kind="ExternalInput")
    out_d = nc.dram_tensor("out", [128, 4 * 132], f32, kind="ExternalOutput")

    al = nc.alloc_sbuf_tensor
    qk = al("qks", [128, QPC + KW], bf16)
    wr = al("wrs", [128, WR_COLS], bf16)
    xs = al("xss", [128, XS_COLS], bf16)
    bat = al("bats", [128, 2], f32)
    arg = [al(f"arg{i}", [128, MAXB * 512], bf16) for i in range(3)]
    th = [al(f"th{i}", [128, MAXB * 512], bf16) for i in range(3)]
    cs0 = al("cs0", [128, QPC], bf16)
    cs1 = al("cs1", [128, QPC], bf16)
    cfin = al("cfin", [128, QPC], bf16)
    el = al("els", [128, QPC], bf16)
    ov = al("ovs", [128, 4 * 132], f32)

    ap = nc.alloc_psum_tensor
    spB = ap("spB", [128, QPC], f32)
    bka = ap("bka", [128, QPC], f32)
    bkb = ap("bkb", [128, QPC], f32)
    vp = [ap(f"vp{i}", [128, 132], f32) for i in range(4)]

    sem = nc.alloc_semaphore
    (sINQ, sINW, sINX, sINB, sADD, sTANH, sMM, sSE, sSH, sXD,
     sEXP, sTRI, sVMM, sEPI, sOUT) = (
        sem(n) for n in ("sINQ", "sINW", "sINX", "sINB", "sADD", "sTANH",
                         "sMM", "sSE", "sSH", "sXD",
                         "sEXP", "sTRI", "sVMM", "sEPI", "sOUT"))

    AP = bass.AP
    QKW = QPC + KW

    def q4ap(s):
        return AP(qk, 0, [[QKW, 128], [0, SUB_BLOCKS[s]], [1, QPC]])

    def k4ap(s):
        return AP(qk, QPC + 4 * SUB_START[s],
                  [[QKW, 128], [4, SUB_BLOCKS[s]], [1, QPC]])

    # stage-A col class v: cols {i : i%16 == v}
    def clsA(t, off):
        return AP(t, off, [[QPC, 128], [16, 32]])

    # exitA quad c: bka col 32v+a (a = 8c+w) -> cs1 col 128c+16w+v
    def exAsrc(c):
        return AP(bka, 8 * c, [[QPC, 128], [32, 16], [1, 8]])

    def exAdst(c):
        return AP(cs1, 128 * c, [[QPC, 128], [1, 16], [16, 8]])

    with nc.Block() as block:

        @block.sync
        def _(sync):
            sync.dma_start(qk[:, :], qk_d[:, :]).then_inc(sINQ, 16)
            sync.dma_start(wr[:, :], wr_d[:, :]).then_inc(sINW, 16)
            sync.dma_start(xs[:, :], xs_d[:, :]).then_inc(sINX, 16)
            sync.dma_start(bat[:, :], ba_d[:, :]).then_inc(sINB, 16)
            sync.wait_ge(sEPI, 2)
            sync.dma_start(out_d[:, 0:264], ov[:, 0:264]).then_inc(sOUT, 16)
            sync.wait_ge(sEPI, 4)
            sync.dma_start(out_d[:, 264:528],
                           ov[:, 264:528]).then_inc(sOUT, 16)

        @block.vector
        def _(vector):
            for s in range(NSUB):
                if s == 0:
                    vector.wait_ge(sINQ, 16)
                if s >= 3:
                    vector.wait_ge(sTANH, s - 2)   # arg[s%3] free
                vector.tensor_add(AP(arg[s % 3], 0,
                                     [[MAXB * 512, 128],
                                      [512, SUB_BLOCKS[s]], [1, 512]]),
                                  q4ap(s), k4ap(s)).then_inc(sADD, 1)
            # shear stage-A exit: one full-width permuted copy
            vector.wait_ge(sSH, 16)
            vector.tensor_copy(AP(cs1, 0, [[QPC, 128], [1, 16], [16, 32]]),
                               AP(bka, 0, [[QPC, 128], [32, 16], [1, 32]])
                               ).then_inc(sXD, 1)
            # triangle el = cfin * M, epilogue interleaved per quad
            for t in range(4):
                if t % 2 == 0:
                    vector.wait_ge(sEXP, t // 2 + 1)
                vector.tensor_tensor(el[:, 128 * t:128 * (t + 1)],
                                     cfin[:, 128 * t:128 * (t + 1)],
                                     wr[:, TRI_OFF:TRI_OFF + 128],
                                     op=mybir.AluOpType.mult).then_inc(sTRI, 1)
                if t >= 1:
                    vector.wait_ge(sVMM, t)
                    vector.tensor_copy(ov[:, 132 * (t - 1):132 * t],
                                       vp[t - 1][:, :]).then_inc(sEPI, 1)
            vector.wait_ge(sVMM, 4)
            vector.tensor_copy(ov[:, 396:528], vp[3][:, :]).then_inc(sEPI, 1)

        @block.scalar
        def _(scalar):
            for s in range(NSUB):
                scalar.wait_ge(sADD, s + 1)
                if s >= 3:
                    scalar.wait_ge(sMM, CUM_BLOCKS[s - 3])   # th[s%3] free
                w = 512 * SUB_BLOCKS[s]
                scalar.activation(th[s % 3][:, :w], arg[s % 3][:, :w],
                                  Tanh).then_inc(sTANH, 1)
            # score exit: tanh(0.5*s + 0.5*ba) = 2*sigmoid(s+ba) - 1
            scalar.wait_ge(sMM, 32)
            scalar.wait_ge(sINB, 16)
            scalar.activation(cs0[:, :], spB[:, :], Tanh,
                              bias=bat[:, 0:1], scale=0.5).then_inc(sSE, 1)
            # exp(0.5*x + 0.5) = exp(sigmoid) in halves; all of stage B
            # must be done first (same-bank concurrent access aborts)
            scalar.wait_ge(sSH, 48)
            for h in range(2):
                scalar.activation(cfin[:, 256 * h:256 * (h + 1)],
                                  bkb[:, 256 * h:256 * (h + 1)], Exp,
                                  bias=bat[:, 1:2], scale=0.5).then_inc(sEXP, 1)

        @block.tensor
        def _(tensor):
            tensor.wait_ge(sINW, 16)
            for j in range(32):
                s = next(i for i in range(NSUB) if j < CUM_BLOCKS[i])
                dgl = j - SUB_START[s]
                if dgl == 0:
                    tensor.wait_ge(sTANH, s + 1)
                v = j % 16
                r = j // 16
                tensor.matmul(spB[64 * r:64 * (r + 1), :],
                              wr[:, 64 * v:64 * (v + 1)],
                              th[s % 3][:, 512 * dgl:512 * (dgl + 1)],
                              start=(v == 0), stop=(v == 15)).then_inc(sMM, 1)
            # shear stage A: rotate col class v by v; contiguous psum block
            tensor.wait_ge(sSE, 1)
            for v in range(16):
                tensor.matmul(bka[:, 32 * v:32 * (v + 1)],
                              wr[:, W64_COLS + 128 * v:W64_COLS + 128 * (v + 1)],
                              clsA(cs0, v), start=True,
                              stop=True).then_inc(sSH, 1)
            # shear stage B per quad: rotate class w by 16w; all contiguous
            tensor.wait_ge(sXD, 1)
            for c in range(4):
                for w in range(8):
                    ri = w + 15 if w > 0 else 0      # R_16w slab index
                    off = 128 * c + 16 * w
                    tensor.matmul(bkb[:, off:off + 16],
                                  wr[:, W64_COLS + 128 * ri:W64_COLS + 128 * (ri + 1)],
                                  cs1[:, off:off + 16], start=True,
                                  stop=True).then_inc(sSH, 1)
            # v matmuls: vp[t] = El_t.T @ XD[t] + C_t.T @ X[t+1]
            tensor.wait_ge(sINX, 16)
            for t in range(4):
                tensor.wait_ge(sTRI, t + 1)
                tensor.matmul(vp[t][:, :],
                              el[:, 128 * t:128 * (t + 1)],
                              xs[:, 660 + 132 * t:660 + 132 * (t + 1)],
                              start=True, stop=False)
                tensor.matmul(vp[t][:, :],
                              cfin[:, 128 * t:128 * (t + 1)],
                              xs[:, 132 * (t + 1):132 * (t + 2)],
                              start=False, stop=True).then_inc(sVMM, 1)

        @block.gpsimd
        def _(gpsimd):
            gpsimd.wait_ge(sOUT, 32)

    nc.finalize()
    return nc


def _prep_inputs(x, mask, Wt, Wx, bh, Wa, ba):
    """Build the 8 per-core input maps (host-side sharding + projections)."""
    x64 = x.astype(np.float64)

    # W64 lhsT variants: variant v maps partition 32*dm+u -> out 4v+dm
    w64 = np.zeros((128, W64_COLS), np.float32)
    for v in range(16):
        for dm in range(4):
            w64[32 * dm:32 * (dm + 1), 64 * v + 4 * v + dm] = Wa[:, 0]
    # rotation matrices R_sh[p, m] = 1 iff m == (p + sh) % 128
    rot = np.zeros((128, ROT_COLS), np.float32)
    m = np.arange(128)
    for ri, sh in enumerate(ROTS):
        rot[(m - sh) % 128, 128 * ri + m] = 1.0
    # lower-triangle mask M[p, b] = 1 iff p >= b
    tri = (np.arange(128)[:, None] >= np.arange(128)[None, :]).astype(np.float32)
    wr = np.concatenate([w64, rot, tri], axis=1).astype(BF16)

    in_maps = []
    for c in range(NCORES):
        b = c // 4
        qs = (c % 4) * QPC
        q = (x64[b] @ Wt.astype(np.float64) + bh.astype(np.float64))
        k = (x64[b] @ Wx.astype(np.float64))
        qT = q[qs:qs + QPC].T.astype(np.float32)          # [32, 512]
        q4 = np.tile(qT, (4, 1))                          # [128, 512]
        lo = qs - 64
        s0, s1 = max(0, lo), min(L, lo + NKEY)
        kx = np.zeros((NKEY + 3, U), np.float64)
        kx[s0 - lo:s1 - lo] = k[s0:s1]
        K4 = np.zeros((128, KW), np.float32)
        for dm in range(4):
            K4[32 * dm:32 * (dm + 1), :] = kx[dm:dm + KW].T
        qkm = np.concatenate([q4, K4], axis=1).astype(BF16)

        mk = mask[b].astype(np.float32)
        xr = np.zeros((NKEY, F), np.float32)
        xr[s0 - lo:s1 - lo] = x[b, s0:s1] * mk[s0:s1, None]
        val = np.zeros(NKEY, np.float32)
        val[s0 - lo:s1 - lo] = mk[s0:s1]
        Xe = np.zeros((NKEY, 132), np.float32)
        Xe[:, :F] = xr
        Xe[:, F] = val
        xcols = [Xe[128 * t:128 * (t + 1)] for t in range(5)]
        xdcols = [xcols[t] - xcols[t + 1] for t in range(4)]
        xsl = np.concatenate(xcols + xdcols, axis=1).astype(BF16)

        bt = np.zeros((128, 2), np.float32)
        bt[:, 0] = 0.5 * float(ba[0])
        bt[:, 1] = 0.5
        in_maps.append({"qk": qkm, "wr": wr, "xs": xsl, "bat": bt})
    return in_maps


def kernel(x, mask, Wt, Wx, bh, Wa, ba, _want_results=False):
    global _built
    from concourse.bass_utils import run_bass_kernel_spmd
    x = np.asarray(x)
    mask = np.asarray(mask)
    Wt, Wx, bh, Wa, ba = (np.asarray(a) for a in (Wt, Wx, bh, Wa, ba))
    if _built is None:
        _built = _build()
    nc = _built
    in_maps = _prep_inputs(x, mask, Wt, Wx, bh, Wa, ba)
    res = run_bass_kernel_spmd(nc, in_maps, core_ids=list(range(NCORES)))
    v = np.zeros((B, L, F), np.float32)
    for c in range(NCORES):
        b = c // 4
        qs = (c % 4) * QPC
        o = res.results[c]["out"]                    # [128, 528]
        for t in range(4):
            blk = o[:, 132 * t:132 * (t + 1)]
            v[b, qs + 128 * t:qs + 128 * (t + 1)] = \
                blk[:, :F] / (blk[:, F:F + 1] + EPS)
    v *= mask.astype(np.float32)[:, :, None]
    if _want_results:
        return v, res
    return v


# revision 32
# speedup vs baseline: 1.0397x; 1.0069x over previous
"""Banded additive attention (width-128) on 8 TRN2 NeuronCores — raw Bass.

Problem: B=2, L=2048, F=128, U=32, WIDTH=128
  q = x@Wt + bh, k = x@Wx
  s_ij = Wa . tanh(q_i + k_j) + ba            (j in [i-64, i+63])
  e_ij = exp(sigmoid(s_ij)) * band * mask
  v_i  = sum_j e_ij x_j / (sum_j e_ij + 1e-7)

Sharding: core c handles batch c//4, queries [(c%4)*512, +512).  No
collectives.  Raw Bass; all synchronization is explicit standalone
wait_ge with hand-counted thresholds (walrus here rejects >1 sem wait
per instruction).

Per-core pipeline (partition p = 32*dm + u; block j in [0,32) covers
d = 4j+dm in [0,128); col i in [0,512) is the query):
  DVE  : arg[p,(j,i)] = q4[p,i] + K4[p, i+4j]      (bf16, 5 subchunks)
  ACT  : tanh(arg)                                 (the bulk)
  PE   : spB[64(j//16)+4(j%16)+dm, i] += block-diag W64 contraction
         over (dm,u) — all 32 matmuls accumulate into ONE psum bank
         (two 64-partition regions), so no placement DMAs and a single
         full-width exit.
  ACT  : cs0 = tanh(0.5*spB + 0.5*ba)  (= 2*sigmoid(s+ba)-1)
  PE   : 2-stage circular shear C[c,i] = sg[(c-i)%128, i]: stage A
         rotates by i%16 into bka laid out col=32v+a (16 contiguous
         col-class matmuls; the interpreter's psum pending-zero model
         requires contiguous matmul outputs).  Per-quad exits undo the
         permutation (2 on ACT, 2 on DVE), then stage B rotates by
         16*((i//16)%8) as 8 tiny contiguous matmuls per quad.
  ACT  : per-quad exp(0.5*x+0.5) = exp(sigmoid) -> cfin.
  DVE  : el quad = cfin quad * M (lower-triangle mask, c>=b); the
         complementary upper part never materializes because
         v = El.T @ (X[t]-X[t+1]) + C.T @ X[t+1]   (El+Eh = C exactly)
  PE   : vp[t] = El.T @ XD[t] + C_t.T @ X[t+1], four separate psum
         banks (a DVE read of one bank while PE accumulates another
         region of the SAME bank aborts the axon emulator); X carries
         a validity column so the denominator falls out of the matmul.
  DVE  : copy vp[t] -> ov slab;  two paired output DMAs;  host divides.
"""

import numpy as np
import ml_dtypes

B, L, F, U = 2, 2048, 128, 32
WIDTH = 128
EPS = 1e-7
NCORES = 8
QPC = (B * L) // NCORES          # 512 queries per core
NKEY = QPC + WIDTH               # 640 key rows per core
KW = NKEY                        # K4 sbuf width
BF16 = ml_dtypes.bfloat16

# subchunk partition of the 32 d-blocks (block = 4 consecutive d)
SUB_BLOCKS = [5, 7, 8, 8, 3, 1]
NSUB = len(SUB_BLOCKS)
SUB_START = [sum(SUB_BLOCKS[:i]) for i in range(NSUB)]
CUM_BLOCKS = [sum(SUB_BLOCKS[:i + 1]) for i in range(NSUB)]
MAXB = max(SUB_BLOCKS)

# shear rotation amounts: stage A = i%16, stage B = 16*((i//16)%8)
ROT_A = list(range(16))                 # 16 matrices (incl. identity)
ROT_B = [16 * w for w in range(1, 8)]   # 7 more (16..112); w=0 reuses R_0
ROTS = ROT_A + ROT_B                    # 23 matrices in the wr slab

W64_COLS = 16 * 64                      # 16 lhsT variants [128, 64]
ROT_COLS = len(ROTS) * 128
TRI_OFF = W64_COLS + ROT_COLS           # lower-triangle mask M
WR_COLS = TRI_OFF + 128
XS_COLS = 5 * 132 + 4 * 132             # X[0..4] then XD[0..3]

_built = None


def _build():
    import concourse.bass as bass
    import concourse.mybir as mybir

    f32 = mybir.dt.float32
    bf16 = mybir.dt.bfloat16
    Tanh = mybir.ActivationFunctionType.Tanh
    Exp = mybir.ActivationFunctionType.Exp
    Copy = mybir.ActivationFunctionType.Copy

    nc = bass.Bass()

    qk_d = nc.dram_tensor("qk", [128, QPC + KW], bf16, kind="ExternalInput")
    wr_d = nc.dram_tensor("wr", [128, WR_COLS], bf16, kind="ExternalInput")
    xs_d = nc.dram_tensor("xs", [128, XS_COLS], bf16, kind="ExternalInput")
    ba_d = nc.dram_tensor("bat", [128, 2], f32, kind="ExternalInput")
    out_d = nc.dram_tensor("out", [128, 4 * 132], f32, kind="ExternalOutput")

    al = nc.alloc_sbuf_tensor
    qk = al("qks", [128, QPC + KW], bf16)
    wr = al("wrs", [128, WR_COLS], bf16)
    xs = al("xss", [128, XS_COLS], bf16)
    bat = al("bats", [128, 2], f32)
    arg = [al(f"arg{i}", [128, MAXB * 512], bf16) for i in range(3)]
    th = [al(f"th{i}", [128, MAXB * 512], bf16) for i in range(3)]
    cs0 = al("cs0", [128, QPC], bf16)
    cs1 = al("cs1", [128, QPC], bf16)
    cfin = al("cfin", [128, QPC], bf16)
    el = al("els", [128, QPC], bf16)
    ov = al("ovs", [128, 4 * 132], f32)

    ap = nc.alloc_psum_tensor
    spB = ap("spB", [128, QPC], f32)
    bka = ap("bka", [128, QPC], f32)
    bkb = ap("bkb", [128, QPC], f32)
    vp = [ap(f"vp{i}", [128, 132], f32) for i in range(4)]

    sem = nc.alloc_semaphore
    (sINQ, sINW, sINX, sINB, sADD, sTANH, sMM, sSE, sSH, sXD,
     sEXP, sTRI, sVMM, sEPI, sOUT) = (
        sem(n) for n in ("sINQ", "sINW", "sINX", "sINB", "sADD", "sTANH",
                         "sMM", "sSE", "sSH", "sXD",
                         "sEXP", "sTRI", "sVMM", "sEPI", "sOUT"))

    AP = bass.AP
    QKW = QPC + KW

    def q4ap(s):
        return AP(qk, 0, [[QKW, 128], [0, SUB_BLOCKS[s]], [1, QPC]])

    def k4ap(s):
        return AP(qk, QPC + 4 * SUB_START[s],
                  [[QKW, 128], [4, SUB_BLOCKS[s]], [1, QPC]])

    # stage-A col class v: cols {i : i%16 == v}
    def clsA(t, off):
        return AP(t, off, [[QPC, 128], [16, 32]])

    # exitA quad c: bka col 32v+a (a = 8c+w) -> cs1 col 128c+16w+v
    def exAsrc(c):
        return AP(bka, 8 * c, [[QPC, 128], [32, 16], [1, 8]])

    def exAdst(c):
        return AP(cs1, 128 * c, [[QPC, 128], [1, 16], [16, 8]])

    with nc.Block() as block:

        @block.sync
        def _(sync):
            sync.dma_start(qk[:, :], qk_d[:, :]).then_inc(sINQ, 16)
            sync.dma_start(wr[:, :], wr_d[:, :]).then_inc(sINW, 16)
            sync.dma_start(xs[:, :], xs_d[:, :]).then_inc(sINX, 16)
            sync.dma_start(bat[:, :], ba_d[:, :]).then_inc(sINB, 16)
            sync.wait_ge(sEPI, 2)
            sync.dma_start(out_d[:, 0:264], ov[:, 0:264]).then_inc(sOUT, 16)
            sync.wait_ge(sEPI, 4)
            sync.dma_start(out_d[:, 264:528],
                           ov[:, 264:528]).then_inc(sOUT, 16)

        @block.vector
        def _(vector):
            for s in range(NSUB):
                if s == 0:
                    vector.wait_ge(sINQ, 16)
                if s >= 3:
                    vector.wait_ge(sTANH, s - 2)   # arg[s%3] free
                vector.tensor_add(AP(arg[s % 3], 0,
                                     [[MAXB * 512, 128],
                                      [512, SUB_BLOCKS[s]], [1, 512]]),
                                  q4ap(s), k4ap(s)).then_inc(sADD, 1)
            # shear stage-A exit: one full-width permuted copy
            vector.wait_ge(sSH, 16)
            vector.tensor_copy(AP(cs1, 0, [[QPC, 128], [1, 16], [16, 32]]),
                               AP(bka, 0, [[QPC, 128], [32, 16], [1, 32]])
                               ).then_inc(sXD, 1)
            # triangle: el quad = cfin quad * M  (keep c >= b)
            for t in range(4):
                if t % 2 == 0:
                    vector.wait_ge(sEXP, t // 2 + 1)
                vector.tensor_tensor(el[:, 128 * t:128 * (t + 1)],
                                     cfin[:, 128 * t:128 * (t + 1)],
                                     wr[:, TRI_OFF:TRI_OFF + 128],
                                     op=mybir.AluOpType.mult).then_inc(sTRI, 1)

        @block.scalar
        def _(scalar):
            for s in range(NSUB):
                scalar.wait_ge(sADD, s + 1)
                if s >= 3:
                    scalar.wait_ge(sMM, CUM_BLOCKS[s - 3])   # th[s%3] free
                w = 512 * SUB_BLOCKS[s]
                scalar.activation(th[s % 3][:, :w], arg[s % 3][:, :w],
                                  Tanh).then_inc(sTANH, 1)
            # score exit: tanh(0.5*s + 0.5*ba) = 2*sigmoid(s+ba) - 1
            scalar.wait_ge(sMM, 32)
            scalar.wait_ge(sINB, 16)
            scalar.activation(cs0[:, :], spB[:, :], Tanh,
                              bias=bat[:, 0:1], scale=0.5).then_inc(sSE, 1)
            # exp(0.5*x + 0.5) = exp(sigmoid) in halves; all of stage B
            # must be done first (same-bank concurrent access aborts)
            scalar.wait_ge(sSH, 48)
            for h in range(2):
                scalar.activation(cfin[:, 256 * h:256 * (h + 1)],
                                  bkb[:, 256 * h:256 * (h + 1)], Exp,
                                  bias=bat[:, 1:2], scale=0.5).then_inc(sEXP, 1)
            # epilogue: psum -> ov slab
            for t in range(4):
                scalar.wait_ge(sVMM, t + 1)
                scalar.activation(ov[:, 132 * t:132 * (t + 1)],
                                  vp[t][:, :], Copy).then_inc(sEPI, 1)

        @block.tensor
        def _(tensor):
            tensor.wait_ge(sINW, 16)
            for j in range(32):
                s = next(i for i in range(NSUB) if j < CUM_BLOCKS[i])
                dgl = j - SUB_START[s]
                if dgl == 0:
                    tensor.wait_ge(sTANH, s + 1)
                v = j % 16
                r = j // 16
                tensor.matmul(spB[64 * r:64 * (r + 1), :],
                              wr[:, 64 * v:64 * (v + 1)],
                              th[s % 3][:, 512 * dgl:512 * (dgl + 1)],
                              start=(v == 0), stop=(v == 15)).then_inc(sMM, 1)
            # shear stage A: rotate col class v by v; contiguous psum block
            tensor.wait_ge(sSE, 1)
            for v in range(16):
                tensor.matmul(bka[:, 32 * v:32 * (v + 1)],
                              wr[:, W64_COLS + 128 * v:W64_COLS + 128 * (v + 1)],
                              clsA(cs0, v), start=True,
                              stop=True).then_inc(sSH, 1)
            # shear stage B per quad: rotate class w by 16w; all contiguous
            tensor.wait_ge(sXD, 1)
            for c in range(4):
                for w in range(8):
                    ri = w + 15 if w > 0 else 0      # R_16w slab index
                    off = 128 * c + 16 * w
                    tensor.matmul(bkb[:, off:off + 16],
                                  wr[:, W64_COLS + 128 * ri:W64_COLS + 128 * (ri + 1)],
                                  cs1[:, off:off + 16], start=True,
                                  stop=True).then_inc(sSH, 1)
            # v matmuls: vp[t] = El_t.T @ XD[t] + C_t.T @ X[t+1]
            tensor.wait_ge(sINX, 16)
            for t in range(4):
                tensor.wait_ge(sTRI, t + 1)
                tensor.matmul(vp[t][:, :],
                              el[:, 128 * t:128 * (t + 1)],
                              xs[:, 660 + 132 * t:660 + 132 * (t + 1)],
                              start=True, stop=False)
                tensor.matmul(vp[t][:, :],
                              cfin[:, 128 * t:128 * (t + 1)],
                              xs[:, 132 * (t + 1):132 * (t + 2)],
                              start=False, stop=True).then_inc(sVMM, 1)

        @block.gpsimd
        def _(gpsimd):
            gpsimd.wait_ge(sOUT, 32)

    nc.finalize()
    return nc


def _prep_inputs(x, mask, Wt, Wx, bh, Wa, ba):
    """Build the 8 per-core input maps (host-side sharding + projections)."""
    x64 = x.astype(np.float64)

    # W64 lhsT variants: variant v maps partition 32*dm+u -> out 4v+dm
    w64 = np.zeros((128, W64_COLS), np.float32)
    for v in range(16):
        for dm in range(4):
            w64[32 * dm:32 * (dm + 1), 64 * v + 4 * v + dm] = Wa[:, 0]
    # rotation matrices R_sh[p, m] = 1 iff m == (p + sh) % 128
    rot = np.zeros((128, ROT_COLS), np.float32)
    m = np.arange(128)
    for ri, sh in enumerate(ROTS):
        rot[(m - sh) % 128, 128 * ri + m] = 1.0
    # lower-triangle mask M[p, b] = 1 iff p >= b
    tri = (np.arange(128)[:, None] >= np.arange(128)[None, :]).astype(np.float32)
    wr = np.concatenate([w64, rot, tri], axis=1).astype(BF16)

    in_maps = []
    for c in range(NCORES):
        b = c // 4
        qs = (c % 4) * QPC
        q = (x64[b] @ Wt.astype(np.float64) + bh.astype(np.float64))
        k = (x64[b] @ Wx.astype(np.float64))
        qT = q[qs:qs + QPC].T.astype(np.float32)          # [32, 512]
        q4 = np.tile(qT, (4, 1))                          # [128, 512]
        lo = qs - 64
        s0, s1 = max(0, lo), min(L, lo + NKEY)
        kx = np.zeros((NKEY + 3, U), np.float64)
        kx[s0 - lo:s1 - lo] = k[s0:s1]
        K4 = np.zeros((128, KW), np.float32)
        for dm in range(4):
            K4[32 * dm:32 * (dm + 1), :] = kx[dm:dm + KW].T
        qkm = np.concatenate([q4, K4], axis=1).astype(BF16)

        mk = mask[b].astype(np.float32)
        xr = np.zeros((NKEY, F), np.float32)
        xr[s0 - lo:s1 - lo] = x[b, s0:s1] * mk[s0:s1, None]
        val = np.zeros(NKEY, np.float32)
        val[s0 - lo:s1 - lo] = mk[s0:s1]
        Xe = np.zeros((NKEY, 132), np.float32)
        Xe[:, :F] = xr
        Xe[:, F] = val
        xcols = [Xe[128 * t:128 * (t + 1)] for t in range(5)]
        xdcols = [xcols[t] - xcols[t + 1] for t in range(4)]
        xsl = np.concatenate(xcols + xdcols, axis=1).astype(BF16)

        bt = np.zeros((128, 2), np.float32)
        bt[:, 0] = 0.5 * float(ba[0])
        bt[:, 1] = 0.5
        in_maps.append({"qk": qkm, "wr": wr, "xs": xsl, "bat": bt})
    return in_maps


def kernel(x, mask, Wt, Wx, bh, Wa, ba, _want_results=False):
    global _built
    from concourse.bass_utils import run_bass_kernel_spmd
    x = np.asarray(x)
    mask = np.asarray(mask)
    Wt, Wx, bh, Wa, ba = (np.asarray(a) for a in (Wt, Wx, bh, Wa, ba))
    if _built is None:
        _built = _build()
    nc = _built
    in_maps = _prep_inputs(x, mask, Wt, Wx, bh, Wa, ba)
    res = run_bass_kernel_spmd(nc, in_maps, core_ids=list(range(NCORES)))
    v = np.zeros((B, L, F), np.float32)
    for c in range(NCORES):
        b = c // 4
        qs = (c % 4) * QPC
        o = res.results[c]["out"]                    # [128, 528]
        for t in range(4):
            blk = o[:, 132 * t:132 * (t + 1)]
            v[b, qs + 128 * t:qs + 128 * (t + 1)] = \
                blk[:, :F] / (blk[:, F:F + 1] + EPS)
    v *= mask.astype(np.float32)[:, :, None]
    if _want_results:
        return v, res
    return v


# revision 33
# speedup vs baseline: 1.0840x; 1.0426x over previous
"""Banded additive attention (width-128) on 8 TRN2 NeuronCores — raw Bass.

Problem: B=2, L=2048, F=128, U=32, WIDTH=128
  q = x@Wt + bh, k = x@Wx
  s_ij = Wa . tanh(q_i + k_j) + ba            (j in [i-64, i+63])
  e_ij = exp(sigmoid(s_ij)) * band * mask
  v_i  = sum_j e_ij x_j / (sum_j e_ij + 1e-7)

Sharding: core c handles batch c//4, queries [(c%4)*512, +512).  No
collectives.  Raw Bass; all synchronization is explicit standalone
wait_ge with hand-counted thresholds (walrus here rejects >1 sem wait
per instruction).

Per-core pipeline (partition p = 32*dm + u; block j in [0,32) covers
d = 4j+dm in [0,128); col i in [0,512) is the query):
  DVE  : arg[p,(j,i)] = q4[p,i] + K4[p, i+4j]      (bf16, 5 subchunks)
  ACT  : tanh(arg)                                 (the bulk)
  PE   : spB[64(j//16)+4(j%16)+dm, i] += block-diag W64 contraction
         over (dm,u) — all 32 matmuls accumulate into ONE psum bank
         (two 64-partition regions), so no placement DMAs and a single
         full-width exit.
  ACT  : cs0 = tanh(0.5*spB + 0.5*ba)  (= 2*sigmoid(s+ba)-1)
  PE   : 2-stage circular shear C[c,i] = sg[(c-i)%128, i]: stage A
         rotates by i%16 into bka laid out col=32v+a (16 contiguous
         col-class matmuls; the interpreter's psum pending-zero model
         requires contiguous matmul outputs).  Per-quad exits undo the
         permutation (2 on ACT, 2 on DVE), then stage B rotates by
         16*((i//16)%8) as 8 tiny contiguous matmuls per quad.
  ACT  : per-quad exp(0.5*x+0.5) = exp(sigmoid) -> cfin.
  DVE  : el quad = cfin quad * M (lower-triangle mask, c>=b); the
         complementary upper part never materializes because
         v = El.T @ (X[t]-X[t+1]) + C.T @ X[t+1]   (El+Eh = C exactly)
  PE   : vp[t] = El.T @ XD[t] + C_t.T @ X[t+1], four separate psum
         banks (a DVE read of one bank while PE accumulates another
         region of the SAME bank aborts the axon emulator); X carries
         a validity column so the denominator falls out of the matmul.
  DVE  : copy vp[t] -> ov slab;  two paired output DMAs;  host divides.
"""

import numpy as np
import ml_dtypes

B, L, F, U = 2, 2048, 128, 32
WIDTH = 128
EPS = 1e-7
NCORES = 8
QPC = (B * L) // NCORES          # 512 queries per core
NKEY = QPC + WIDTH               # 640 key rows per core
KW = NKEY                        # K4 sbuf width
BF16 = ml_dtypes.bfloat16

# subchunk partition of the 32 d-blocks (block = 4 consecutive d)
SUB_BLOCKS = [5, 7, 8, 8, 3, 1]
NSUB = len(SUB_BLOCKS)
SUB_START = [sum(SUB_BLOCKS[:i]) for i in range(NSUB)]
CUM_BLOCKS = [sum(SUB_BLOCKS[:i + 1]) for i in range(NSUB)]
MAXB = max(SUB_BLOCKS)

W64_COLS = 16 * 64                      # 16 lhsT variants [128, 64]
TRI_OFF = W64_COLS                      # lower-triangle mask M
WR_COLS = TRI_OFF + 128
ROTF_COLS = 128 * 128                   # all 128 rotation matrices
XS_COLS = 5 * 132 + 4 * 132             # X[0..4] then XD[0..3]

_built = None


def _build():
    import concourse.bass as bass
    import concourse.mybir as mybir

    f32 = mybir.dt.float32
    bf16 = mybir.dt.bfloat16
    Tanh = mybir.ActivationFunctionType.Tanh
    Exp = mybir.ActivationFunctionType.Exp
    Copy = mybir.ActivationFunctionType.Copy

    nc = bass.Bass()

    qk_d = nc.dram_tensor("qk", [128, QPC + KW], bf16, kind="ExternalInput")
    wr_d = nc.dram_tensor("wr", [128, WR_COLS], bf16, kind="ExternalInput")
    rf_d = nc.dram_tensor("rf", [128, ROTF_COLS], bf16, kind="ExternalInput")
    xs_d = nc.dram_tensor("xs", [128, XS_COLS], bf16, kind="ExternalInput")
    ba_d = nc.dram_tensor("bat", [128, 2], f32, kind="ExternalInput")
    out_d = nc.dram_tensor("out", [128, 4 * 132], f32, kind="ExternalOutput")

    al = nc.alloc_sbuf_tensor
    qk = al("qks", [128, QPC + KW], bf16)
    wr = al("wrs", [128, WR_COLS], bf16)
    rf = al("rfs", [128, ROTF_COLS], bf16)
    xs = al("xss", [128, XS_COLS], bf16)
    bat = al("bats", [128, 2], f32)
    arg = [al(f"arg{i}", [128, MAXB * 512], bf16) for i in range(3)]
    th = [al(f"th{i}", [128, MAXB * 512], bf16) for i in range(3)]
    cs0 = al("cs0", [128, QPC], bf16)
    cfin = al("cfin", [128, QPC], bf16)
    el = al("els", [128, QPC], bf16)
    ov = al("ovs", [128, 4 * 132], f32)

    ap = nc.alloc_psum_tensor
    spB = ap("spB", [128, QPC], f32)
    bka = ap("bka", [128, QPC], f32)
    vp = [ap(f"vp{i}", [128, 132], f32) for i in range(4)]

    sem = nc.alloc_semaphore
    (sINQ, sINW, sINX, sINB, sINR, sADD, sTANH, sMM, sSE, sSH,
     sEXP, sTRI, sVMM, sEPI, sOUT) = (
        sem(n) for n in ("sINQ", "sINW", "sINX", "sINB", "sINR", "sADD",
                         "sTANH", "sMM", "sSE", "sSH",
                         "sEXP", "sTRI", "sVMM", "sEPI", "sOUT"))

    AP = bass.AP
    QKW = QPC + KW

    def q4ap(s):
        return AP(qk, 0, [[QKW, 128], [0, SUB_BLOCKS[s]], [1, QPC]])

    def k4ap(s):
        return AP(qk, QPC + 4 * SUB_START[s],
                  [[QKW, 128], [4, SUB_BLOCKS[s]], [1, QPC]])

    with nc.Block() as block:

        @block.sync
        def _(sync):
            sync.dma_start(qk[:, :], qk_d[:, :]).then_inc(sINQ, 16)
            sync.dma_start(wr[:, :], wr_d[:, :]).then_inc(sINW, 16)
            sync.dma_start(xs[:, :], xs_d[:, :]).then_inc(sINX, 16)
            sync.dma_start(bat[:, :], ba_d[:, :]).then_inc(sINB, 16)
            sync.dma_start(rf[:, :], rf_d[:, :]).then_inc(sINR, 16)
            sync.wait_ge(sEPI, 2)
            sync.dma_start(out_d[:, 0:264], ov[:, 0:264]).then_inc(sOUT, 16)
            sync.wait_ge(sEPI, 4)
            sync.dma_start(out_d[:, 264:528],
                           ov[:, 264:528]).then_inc(sOUT, 16)

        @block.vector
        def _(vector):
            for s in range(NSUB):
                if s == 0:
                    vector.wait_ge(sINQ, 16)
                if s >= 3:
                    vector.wait_ge(sTANH, s - 2)   # arg[s%3] free
                vector.tensor_add(AP(arg[s % 3], 0,
                                     [[MAXB * 512, 128],
                                      [512, SUB_BLOCKS[s]], [1, 512]]),
                                  q4ap(s), k4ap(s)).then_inc(sADD, 1)
            # triangle: el quad = cfin quad * M  (keep c >= b)
            for t in range(4):
                if t % 2 == 0:
                    vector.wait_ge(sEXP, t // 2 + 1)
                vector.tensor_tensor(el[:, 128 * t:128 * (t + 1)],
                                     cfin[:, 128 * t:128 * (t + 1)],
                                     wr[:, TRI_OFF:TRI_OFF + 128],
                                     op=mybir.AluOpType.mult).then_inc(sTRI, 1)

        @block.scalar
        def _(scalar):
            for s in range(NSUB):
                scalar.wait_ge(sADD, s + 1)
                if s >= 3:
                    scalar.wait_ge(sMM, CUM_BLOCKS[s - 3])   # th[s%3] free
                w = 512 * SUB_BLOCKS[s]
                scalar.activation(th[s % 3][:, :w], arg[s % 3][:, :w],
                                  Tanh).then_inc(sTANH, 1)
            # score exit: tanh(0.5*s + 0.5*ba) = 2*sigmoid(s+ba) - 1
            scalar.wait_ge(sMM, 32)
            scalar.wait_ge(sINB, 16)
            scalar.activation(cs0[:, :], spB[:, :], Tanh,
                              bias=bat[:, 0:1], scale=0.5).then_inc(sSE, 1)
            # exp(0.5*x + 0.5) = exp(sigmoid) in halves from the permuted
            # shear bank (col' = 4b + t); the whole stage must be done first
            # (same-bank concurrent access aborts)
            scalar.wait_ge(sSH, 128)
            for h in range(2):
                scalar.activation(
                    AP(cfin, 256 * h, [[QPC, 128], [1, 128], [128, 2]]),
                    AP(bka, 2 * h, [[QPC, 128], [4, 128], [1, 2]]),
                    Exp, bias=bat[:, 1:2], scale=0.5).then_inc(sEXP, 1)
            # epilogue: psum -> ov slab
            for t in range(4):
                scalar.wait_ge(sVMM, t + 1)
                scalar.activation(ov[:, 132 * t:132 * (t + 1)],
                                  vp[t][:, :], Copy).then_inc(sEPI, 1)

        @block.tensor
        def _(tensor):
            tensor.wait_ge(sINW, 16)
            for j in range(32):
                s = next(i for i in range(NSUB) if j < CUM_BLOCKS[i])
                dgl = j - SUB_START[s]
                if dgl == 0:
                    tensor.wait_ge(sTANH, s + 1)
                v = j % 16
                r = j // 16
                tensor.matmul(spB[64 * r:64 * (r + 1), :],
                              wr[:, 64 * v:64 * (v + 1)],
                              th[s % 3][:, 512 * dgl:512 * (dgl + 1)],
                              start=(v == 0), stop=(v == 15)).then_inc(sMM, 1)
            # single-stage shear: rotate col class {i%128 == b} by b into
            # contiguous psum block [4b, 4b+4) (col' = 4b + t)
            tensor.wait_ge(sSE, 1)
            tensor.wait_ge(sINR, 16)
            for b in range(128):
                tensor.matmul(bka[:, 4 * b:4 * (b + 1)],
                              rf[:, 128 * b:128 * (b + 1)],
                              AP(cs0, b, [[QPC, 128], [128, 4]]), start=True,
                              stop=True).then_inc(sSH, 1)
            # v matmuls: vp[t] = El_t.T @ XD[t] + C_t.T @ X[t+1]
            tensor.wait_ge(sINX, 16)
            for t in range(4):
                tensor.wait_ge(sTRI, t + 1)
                tensor.matmul(vp[t][:, :],
                              el[:, 128 * t:128 * (t + 1)],
                              xs[:, 660 + 132 * t:660 + 132 * (t + 1)],
                              start=True, stop=False)
                tensor.matmul(vp[t][:, :],
                              cfin[:, 128 * t:128 * (t + 1)],
                              xs[:, 132 * (t + 1):132 * (t + 2)],
                              start=False, stop=True).then_inc(sVMM, 1)

        @block.gpsimd
        def _(gpsimd):
            gpsimd.wait_ge(sOUT, 32)

    nc.finalize()
    return nc


def _prep_inputs(x, mask, Wt, Wx, bh, Wa, ba):
    """Build the 8 per-core input maps (host-side sharding + projections)."""
    x64 = x.astype(np.float64)

    # W64 lhsT variants: variant v maps partition 32*dm+u -> out 4v+dm
    w64 = np.zeros((128, W64_COLS), np.float32)
    for v in range(16):
        for dm in range(4):
            w64[32 * dm:32 * (dm + 1), 64 * v + 4 * v + dm] = Wa[:, 0]
    # full rotation slab: R_b[p, m] = 1 iff m == (p + b) % 128
    rotf = np.zeros((128, ROTF_COLS), np.float32)
    m = np.arange(128)
    for b in range(128):
        rotf[(m - b) % 128, 128 * b + m] = 1.0
    rotf = rotf.astype(BF16)
    # lower-triangle mask M[p, b] = 1 iff p >= b
    tri = (np.arange(128)[:, None] >= np.arange(128)[None, :]).astype(np.float32)
    wr = np.concatenate([w64, tri], axis=1).astype(BF16)

    in_maps = []
    for c in range(NCORES):
        b = c // 4
        qs = (c % 4) * QPC
        q = (x64[b] @ Wt.astype(np.float64) + bh.astype(np.float64))
        k = (x64[b] @ Wx.astype(np.float64))
        qT = q[qs:qs + QPC].T.astype(np.float32)          # [32, 512]
        q4 = np.tile(qT, (4, 1))                          # [128, 512]
        lo = qs - 64
        s0, s1 = max(0, lo), min(L, lo + NKEY)
        kx = np.zeros((NKEY + 3, U), np.float64)
        kx[s0 - lo:s1 - lo] = k[s0:s1]
        K4 = np.zeros((128, KW), np.float32)
        for dm in range(4):
            K4[32 * dm:32 * (dm + 1), :] = kx[dm:dm + KW].T
        qkm = np.concatenate([q4, K4], axis=1).astype(BF16)

        mk = mask[b].astype(np.float32)
        xr = np.zeros((NKEY, F), np.float32)
        xr[s0 - lo:s1 - lo] = x[b, s0:s1] * mk[s0:s1, None]
        val = np.zeros(NKEY, np.float32)
        val[s0 - lo:s1 - lo] = mk[s0:s1]
        Xe = np.zeros((NKEY, 132), np.float32)
        Xe[:, :F] = xr
        Xe[:, F] = val
        xcols = [Xe[128 * t:128 * (t + 1)] for t in range(5)]
        xdcols = [xcols[t] - xcols[t + 1] for t in range(4)]
        xsl = np.concatenate(xcols + xdcols, axis=1).astype(BF16)

        bt = np.zeros((128, 2), np.float32)
        bt[:, 0] = 0.5 * float(ba[0])
        bt[:, 1] = 0.5
        in_maps.append({"qk": qkm, "wr": wr, "rf": rotf, "xs": xsl,
                        "bat": bt})
    return in_maps


def kernel(x, mask, Wt, Wx, bh, Wa, ba, _want_results=False):
    global _built
    from concourse.bass_utils import run_bass_kernel_spmd
    x = np.asarray(x)
    mask = np.asarray(mask)
    Wt, Wx, bh, Wa, ba = (np.asarray(a) for a in (Wt, Wx, bh, Wa, ba))
    if _built is None:
        _built = _build()
    nc = _built
    in_maps = _prep_inputs(x, mask, Wt, Wx, bh, Wa, ba)
    res = run_bass_kernel_spmd(nc, in_maps, core_ids=list(range(NCORES)))
    v = np.zeros((B, L, F), np.float32)
    for c in range(NCORES):
        b = c // 4
        qs = (c % 4) * QPC
        o = res.results[c]["out"]                    # [128, 528]
        for t in range(4):
            blk = o[:, 132 * t:132 * (t + 1)]
            v[b, qs + 128 * t:qs + 128 * (t + 1)] = \
                blk[:, :F] / (blk[:, F:F + 1] + EPS)
    v *= mask.astype(np.float32)[:, :, None]
    if _want_results:
        return v, res
    return v


# revision 34
# speedup vs baseline: 1.1013x; 1.0159x over previous
"""Banded additive attention (width-128) on 8 TRN2 NeuronCores — raw Bass.

Problem: B=2, L=2048, F=128, U=32, WIDTH=128
  q = x@Wt + bh, k = x@Wx
  s_ij = Wa . tanh(q_i + k_j) + ba            (j in [i-64, i+63])
  e_ij = exp(sigmoid(s_ij)) * band * mask
  v_i  = sum_j e_ij x_j / (sum_j e_ij + 1e-7)

Sharding: core c handles batch c//4, queries [(c%4)*512, +512).  No
collectives.  Raw Bass; all synchronization is explicit standalone
wait_ge with hand-counted thresholds (walrus here rejects >1 sem wait
per instruction).

Per-core pipeline (partition p = 32*dm + u; block j in [0,32) covers
d = 4j+dm in [0,128); col i in [0,512) is the query):
  DVE  : arg[p,(j,i)] = q4[p,i] + K4[p, i+4j]      (bf16, 5 subchunks)
  ACT  : tanh(arg)                                 (the bulk)
  PE   : spB[64(j//16)+4(j%16)+dm, i] += block-diag W64 contraction
         over (dm,u) — all 32 matmuls accumulate into ONE psum bank
         (two 64-partition regions), so no placement DMAs and a single
         full-width exit.
  ACT  : cs0 = tanh(0.5*spB + 0.5*ba)  (= 2*sigmoid(s+ba)-1)
  PE   : 2-stage circular shear C[c,i] = sg[(c-i)%128, i]: stage A
         rotates by i%16 into bka laid out col=32v+a (16 contiguous
         col-class matmuls; the interpreter's psum pending-zero model
         requires contiguous matmul outputs).  Per-quad exits undo the
         permutation (2 on ACT, 2 on DVE), then stage B rotates by
         16*((i//16)%8) as 8 tiny contiguous matmuls per quad.
  ACT  : per-quad exp(0.5*x+0.5) = exp(sigmoid) -> cfin.
  DVE  : el quad = cfin quad * M (lower-triangle mask, c>=b); the
         complementary upper part never materializes because
         v = El.T @ (X[t]-X[t+1]) + C.T @ X[t+1]   (El+Eh = C exactly)
  PE   : vp[t] = El.T @ XD[t] + C_t.T @ X[t+1], four separate psum
         banks (a DVE read of one bank while PE accumulates another
         region of the SAME bank aborts the axon emulator); X carries
         a validity column so the denominator falls out of the matmul.
  DVE  : copy vp[t] -> ov slab;  two paired output DMAs;  host divides.
"""

import numpy as np
import ml_dtypes

B, L, F, U = 2, 2048, 128, 32
WIDTH = 128
EPS = 1e-7
NCORES = 8
QPC = (B * L) // NCORES          # 512 queries per core
NKEY = QPC + WIDTH               # 640 key rows per core
KW = NKEY                        # K4 sbuf width
BF16 = ml_dtypes.bfloat16

# subchunk partition of the 32 d-blocks (block = 4 consecutive d)
SUB_BLOCKS = [2, 3, 7, 8, 8, 3, 1]
NSUB = len(SUB_BLOCKS)
SUB_START = [sum(SUB_BLOCKS[:i]) for i in range(NSUB)]
CUM_BLOCKS = [sum(SUB_BLOCKS[:i + 1]) for i in range(NSUB)]
MAXB = max(SUB_BLOCKS)

W64_COLS = 16 * 64                      # 16 lhsT variants [128, 64]
TRI_OFF = W64_COLS                      # lower-triangle mask M
WR_COLS = TRI_OFF + 128
ROTF_COLS = 128 * 128                   # all 128 rotation matrices
XS_COLS = 5 * 132 + 4 * 132             # X[0..4] then XD[0..3]

_built = None


def _build():
    import concourse.bass as bass
    import concourse.mybir as mybir

    f32 = mybir.dt.float32
    bf16 = mybir.dt.bfloat16
    Tanh = mybir.ActivationFunctionType.Tanh
    Exp = mybir.ActivationFunctionType.Exp
    Copy = mybir.ActivationFunctionType.Copy

    nc = bass.Bass()

    qk_d = nc.dram_tensor("qk", [128, QPC + KW], bf16, kind="ExternalInput")
    wr_d = nc.dram_tensor("wr", [128, WR_COLS], bf16, kind="ExternalInput")
    rf_d = nc.dram_tensor("rf", [128, ROTF_COLS], bf16, kind="ExternalInput")
    xs_d = nc.dram_tensor("xs", [128, XS_COLS], bf16, kind="ExternalInput")
    ba_d = nc.dram_tensor("bat", [128, 2], f32, kind="ExternalInput")
    out_d = nc.dram_tensor("out", [128, 4 * 132], f32, kind="ExternalOutput")

    al = nc.alloc_sbuf_tensor
    qk = al("qks", [128, QPC + KW], bf16)
    wr = al("wrs", [128, WR_COLS], bf16)
    rf = al("rfs", [128, ROTF_COLS], bf16)
    xs = al("xss", [128, XS_COLS], bf16)
    bat = al("bats", [128, 2], f32)
    arg = [al(f"arg{i}", [128, MAXB * 512], bf16) for i in range(3)]
    th = [al(f"th{i}", [128, MAXB * 512], bf16) for i in range(3)]
    cs0 = al("cs0", [128, QPC], bf16)
    cfin = al("cfin", [128, QPC], bf16)
    el = al("els", [128, QPC], bf16)
    ov = al("ovs", [128, 4 * 132], f32)

    ap = nc.alloc_psum_tensor
    spB = ap("spB", [128, QPC], f32)
    bka = ap("bka", [128, QPC], f32)
    vp = [ap(f"vp{i}", [128, 132], f32) for i in range(4)]

    sem = nc.alloc_semaphore
    (sINQ, sINW, sINX, sINB, sINR, sADD, sTANH, sMM, sSE, sSH,
     sEXP, sTRI, sVMM, sEPA, sEPD, sOUT) = (
        sem(n) for n in ("sINQ", "sINW", "sINX", "sINB", "sINR", "sADD",
                         "sTANH", "sMM", "sSE", "sSH",
                         "sEXP", "sTRI", "sVMM", "sEPA", "sEPD", "sOUT"))

    AP = bass.AP
    QKW = QPC + KW

    def q4ap(s):
        return AP(qk, 0, [[QKW, 128], [0, SUB_BLOCKS[s]], [1, QPC]])

    def k4ap(s):
        return AP(qk, QPC + 4 * SUB_START[s],
                  [[QKW, 128], [4, SUB_BLOCKS[s]], [1, QPC]])

    with nc.Block() as block:

        @block.sync
        def _(sync):
            sync.dma_start(qk[:, :], qk_d[:, :]).then_inc(sINQ, 16)
            sync.dma_start(wr[:, :], wr_d[:, :]).then_inc(sINW, 16)
            sync.dma_start(xs[:, :], xs_d[:, :]).then_inc(sINX, 16)
            sync.dma_start(bat[:, :], ba_d[:, :]).then_inc(sINB, 16)
            sync.dma_start(rf[:, :], rf_d[:, :]).then_inc(sINR, 16)
            sync.wait_ge(sEPD, 1)
            sync.wait_ge(sEPA, 1)
            sync.dma_start(out_d[:, 0:264], ov[:, 0:264]).then_inc(sOUT, 16)
            sync.wait_ge(sEPD, 2)
            sync.wait_ge(sEPA, 2)
            sync.dma_start(out_d[:, 264:528],
                           ov[:, 264:528]).then_inc(sOUT, 16)

        @block.vector
        def _(vector):
            for s in range(NSUB):
                if s == 0:
                    vector.wait_ge(sINQ, 16)
                if s >= 3:
                    vector.wait_ge(sTANH, s - 2)   # arg[s%3] free
                vector.tensor_add(AP(arg[s % 3], 0,
                                     [[MAXB * 512, 128],
                                      [512, SUB_BLOCKS[s]], [1, 512]]),
                                  q4ap(s), k4ap(s)).then_inc(sADD, 1)
            # triangle: el quad = cfin quad * M  (keep c >= b)
            for t in range(4):
                if t % 2 == 0:
                    vector.wait_ge(sEXP, t // 2 + 1)
                vector.tensor_tensor(el[:, 128 * t:128 * (t + 1)],
                                     cfin[:, 128 * t:128 * (t + 1)],
                                     wr[:, TRI_OFF:TRI_OFF + 128],
                                     op=mybir.AluOpType.mult).then_inc(sTRI, 1)
            for t in (0, 2):
                vector.wait_ge(sVMM, t + 1)
                vector.tensor_copy(ov[:, 132 * t:132 * (t + 1)],
                                   vp[t][:, :]).then_inc(sEPD, 1)

        @block.scalar
        def _(scalar):
            for s in range(NSUB):
                scalar.wait_ge(sADD, s + 1)
                if s >= 3:
                    scalar.wait_ge(sMM, CUM_BLOCKS[s - 3])   # th[s%3] free
                w = 512 * SUB_BLOCKS[s]
                scalar.activation(th[s % 3][:, :w], arg[s % 3][:, :w],
                                  Tanh).then_inc(sTANH, 1)
            # score exit: tanh(0.5*s + 0.5*ba) = 2*sigmoid(s+ba) - 1
            scalar.wait_ge(sMM, 32)
            scalar.wait_ge(sINB, 16)
            scalar.activation(cs0[:, :], spB[:, :], Tanh,
                              bias=bat[:, 0:1], scale=0.5).then_inc(sSE, 1)
            # exp(0.5*x + 0.5) = exp(sigmoid) in halves from the permuted
            # shear bank (col' = 4b + t); the whole stage must be done first
            # (same-bank concurrent access aborts)
            scalar.wait_ge(sSH, 128)
            for h in range(2):
                scalar.activation(
                    AP(cfin, 256 * h, [[QPC, 128], [1, 128], [128, 2]]),
                    AP(bka, 2 * h, [[QPC, 128], [4, 128], [1, 2]]),
                    Exp, bias=bat[:, 1:2], scale=0.5).then_inc(sEXP, 1)
            # epilogue: psum -> ov slab (odd quads; DVE does even)
            for t in (1, 3):
                scalar.wait_ge(sVMM, t + 1)
                scalar.activation(ov[:, 132 * t:132 * (t + 1)],
                                  vp[t][:, :], Copy).then_inc(sEPA, 1)

        @block.tensor
        def _(tensor):
            tensor.wait_ge(sINW, 16)
            for j in range(32):
                s = next(i for i in range(NSUB) if j < CUM_BLOCKS[i])
                dgl = j - SUB_START[s]
                if dgl == 0:
                    tensor.wait_ge(sTANH, s + 1)
                v = j % 16
                r = j // 16
                tensor.matmul(spB[64 * r:64 * (r + 1), :],
                              wr[:, 64 * v:64 * (v + 1)],
                              th[s % 3][:, 512 * dgl:512 * (dgl + 1)],
                              start=(v == 0), stop=(v == 15)).then_inc(sMM, 1)
            # single-stage shear: rotate col class {i%128 == b} by b into
            # contiguous psum block [4b, 4b+4) (col' = 4b + t)
            tensor.wait_ge(sSE, 1)
            tensor.wait_ge(sINR, 16)
            for b in range(128):
                tensor.matmul(bka[:, 4 * b:4 * (b + 1)],
                              rf[:, 128 * b:128 * (b + 1)],
                              AP(cs0, b, [[QPC, 128], [128, 4]]), start=True,
                              stop=True).then_inc(sSH, 1)
            # v matmuls: vp[t] = El_t.T @ XD[t] + C_t.T @ X[t+1]
            tensor.wait_ge(sINX, 16)
            for t in range(4):
                tensor.wait_ge(sTRI, t + 1)
                tensor.matmul(vp[t][:, :],
                              el[:, 128 * t:128 * (t + 1)],
                              xs[:, 660 + 132 * t:660 + 132 * (t + 1)],
                              start=True, stop=False)
                tensor.matmul(vp[t][:, :],
                              cfin[:, 128 * t:128 * (t + 1)],
                              xs[:, 132 * (t + 1):132 * (t + 2)],
                              start=False, stop=True).then_inc(sVMM, 1)

        @block.gpsimd
        def _(gpsimd):
            gpsimd.wait_ge(sOUT, 32)

    nc.finalize()
    return nc


def _prep_inputs(x, mask, Wt, Wx, bh, Wa, ba):
    """Build the 8 per-core input maps (host-side sharding + projections)."""
    x64 = x.astype(np.float64)

    # W64 lhsT variants: variant v maps partition 32*dm+u -> out 4v+dm
    w64 = np.zeros((128, W64_COLS), np.float32)
    for v in range(16):
        for dm in range(4):
            w64[32 * dm:32 * (dm + 1), 64 * v + 4 * v + dm] = Wa[:, 0]
    # full rotation slab: R_b[p, m] = 1 iff m == (p + b) % 128
    rotf = np.zeros((128, ROTF_COLS), np.float32)
    m = np.arange(128)
    for b in range(128):
        rotf[(m - b) % 128, 128 * b + m] = 1.0
    rotf = rotf.astype(BF16)
    # lower-triangle mask M[p, b] = 1 iff p >= b
    tri = (np.arange(128)[:, None] >= np.arange(128)[None, :]).astype(np.float32)
    wr = np.concatenate([w64, tri], axis=1).astype(BF16)

    in_maps = []
    for c in range(NCORES):
        b = c // 4
        qs = (c % 4) * QPC
        q = (x64[b] @ Wt.astype(np.float64) + bh.astype(np.float64))
        k = (x64[b] @ Wx.astype(np.float64))
        qT = q[qs:qs + QPC].T.astype(np.float32)          # [32, 512]
        q4 = np.tile(qT, (4, 1))                          # [128, 512]
        lo = qs - 64
        s0, s1 = max(0, lo), min(L, lo + NKEY)
        kx = np.zeros((NKEY + 3, U), np.float64)
        kx[s0 - lo:s1 - lo] = k[s0:s1]
        K4 = np.zeros((128, KW), np.float32)
        for dm in range(4):
            K4[32 * dm:32 * (dm + 1), :] = kx[dm:dm + KW].T
        qkm = np.concatenate([q4, K4], axis=1).astype(BF16)

        mk = mask[b].astype(np.float32)
        xr = np.zeros((NKEY, F), np.float32)
        xr[s0 - lo:s1 - lo] = x[b, s0:s1] * mk[s0:s1, None]
        val = np.zeros(NKEY, np.float32)
        val[s0 - lo:s1 - lo] = mk[s0:s1]
        Xe = np.zeros((NKEY, 132), np.float32)
        Xe[:, :F] = xr
        Xe[:, F] = val
        xcols = [Xe[128 * t:128 * (t + 1)] for t in range(5)]
        xdcols = [xcols[t] - xcols[t + 1] for t in range(4)]
        xsl = np.concatenate(xcols + xdcols, axis=1).astype(BF16)

        bt = np.zeros((128, 2), np.float32)
        bt[:, 0] = 0.5 * float(ba[0])
        bt[:, 1] = 0.5
        in_maps.append({"qk": qkm, "wr": wr, "rf": rotf, "xs": xsl,
                        "bat": bt})
    return in_maps


def kernel(x, mask, Wt, Wx, bh, Wa, ba, _want_results=False):
    global _built
    from concourse.bass_utils import run_bass_kernel_spmd
    x = np.asarray(x)
    mask = np.asarray(mask)
    Wt, Wx, bh, Wa, ba = (np.asarray(a) for a in (Wt, Wx, bh, Wa, ba))
    if _built is None:
        _built = _build()
    nc = _built
    in_maps = _prep_inputs(x, mask, Wt, Wx, bh, Wa, ba)
    res = run_bass_kernel_spmd(nc, in_maps, core_ids=list(range(NCORES)))
    v = np.zeros((B, L, F), np.float32)
    for c in range(NCORES):
        b = c // 4
        qs = (c % 4) * QPC
        o = res.results[c]["out"]                    # [128, 528]
        for t in range(4):
            blk = o[:, 132 * t:132 * (t + 1)]
            v[b, qs + 128 * t:qs + 128 * (t + 1)] = \
                blk[:, :F] / (blk[:, F:F + 1] + EPS)
    v *= mask.astype(np.float32)[:, :, None]
    if _want_results:
        return v, res
    return v
